# revision 2
# baseline (speedup 1.0000x reference)
"""DeepseekMoE (moe_routing) Trainium2 kernel.

The v1/v2 kernels are limited by HWDGE ring occupancy (~130 DMAs/rep of
360-512KB at ~150-270 GB/s effective on a single FIFO ring). v3:
- one 2MB DMA per pr for w13 (both slots x both pair panels packed on
  the host into [NPAIR, P, ns, 2, KH, P]);
- one 720KB DMA per m for w2 (both slots packed [MH, P, ns, KI, P]);
- one 1MB DMA per shared pair ([KS, P, 2, KH, P]);
- outputs moved to the second HWDGE ring (ACT engine) and packed into
  ~0.7-1MB stores via partition-major DRAM layouts ([P, MH, T] /
  [P, MH, cap]);
- ACT Silu + single DVE mul; shp psum copies pinned to DVE so the ACT
  ring only carries output DMAs.
"""

import numpy as np
import ml_dtypes

import concourse.mybir as mybir
import concourse.tile as tile
from concourse import bacc
from concourse.bass_utils import run_bass_kernel_spmd

BF16 = ml_dtypes.bfloat16
F32 = np.float32

T, H, E, I = 1024, 2048, 16, 1408
I2 = 2 * I
IS = 2 * I
SSH = 384
TOP_K, N_GROUP, TOPK_GROUP = 4, 4, 2
ROUTED_SCALE = 2.5
N_CORES = 8
P = 128
KH = H // P
KI = I // P
MW = I2 // P
MH = H // P
NPAIR = I // P
KS = SSH // P


def _sigmoid(x):
    return 1.0 / (1.0 + np.exp(-x))


def _route(x, gate_weight, gate_bias):
    logits = x.astype(np.float64) @ gate_weight.astype(np.float64).T
    scores = _sigmoid(logits)
    choice = scores + gate_bias.astype(np.float64)[None, :]
    g = choice.reshape(T, N_GROUP, E // N_GROUP)
    top2sum = np.sort(g, axis=-1)[..., -2:].sum(-1)
    gidx = np.argsort(-top2sum, axis=-1, kind="stable")[:, :TOPK_GROUP]
    gmask = np.zeros((T, N_GROUP), bool)
    gmask[np.arange(T)[:, None], gidx] = True
    emask = np.repeat(gmask, E // N_GROUP, axis=1)
    masked = np.where(emask, choice, -np.inf)
    topk_ids = np.argsort(-masked, axis=-1, kind="stable")[:, :TOP_K]
    topk_w = np.take_along_axis(scores, topk_ids, axis=1)
    topk_w = topk_w / topk_w.sum(-1, keepdims=True) * ROUTED_SCALE
    return topk_ids.astype(np.int32), topk_w


def _silu_mul_np(gu):
    g, u = gu[:, :gu.shape[1] // 2], gu[:, gu.shape[1] // 2:]
    return (g / (1.0 + np.exp(-g))) * u


def _pack_lhs_panels(w, n_m, n_k):
    a = w.reshape(n_m, P, n_k, P)
    return np.ascontiguousarray(a.transpose(0, 3, 2, 1))


def _pack_rhs(xcols):
    a = xcols.reshape(-1, KH, P)
    return np.ascontiguousarray(a.transpose(2, 1, 0))


def _build_program(CB, CS, reps=1):
    nc = bacc.Bacc(None, target_bir_lowering=False)
    bf = mybir.dt.bfloat16
    f32 = mybir.dt.float32

    slot_caps = [c for c in (CB, CS) if c > 0]
    ns = len(slot_caps)

    w13p = nc.dram_tensor("w13p", [NPAIR, P, ns, 2, KH, P], bf,
                          kind="ExternalInput")
    w2p = nc.dram_tensor("w2p", [MH, P, ns, KI, P], bf, kind="ExternalInput")
    xgq = [nc.dram_tensor(f"xgq{s}", [P, KH, slot_caps[s]], bf, kind="ExternalInput")
           for s in range(ns)]
    wtb = [nc.dram_tensor(f"wtb{s}", [P, slot_caps[s]], f32, kind="ExternalInput")
           for s in range(ns)]
    sgup = nc.dram_tensor("sgup", [KS, P, 2, KH, P], bf, kind="ExternalInput")
    sdq = nc.dram_tensor("sdq", [MH, P, KS, P], bf, kind="ExternalInput")
    xtq = nc.dram_tensor("xtq", [P, KH, T], bf, kind="ExternalInput")

    # partition-major outputs so stores pack into multi-panel DMAs
    yout = [nc.dram_tensor(f"y{s}", [P, MH, slot_caps[s]], bf,
                           kind="ExternalOutput") for s in range(ns)]
    shp = nc.dram_tensor("shp", [P, MH, T], bf, kind="ExternalOutput")

    YGRP = 8   # y m-panels per output DMA
    SGRP = 4   # shp m-panels per output DMA

    with tile.TileContext(nc) as tc:
        with (
            tc.tile_pool(name="resident", bufs=1) as res,
            tc.tile_pool(name="wpanel", bufs=2) as wpool,
            tc.tile_pool(name="hbuf", bufs=1) as hpool,
            tc.tile_pool(name="silu", bufs=4) as spool,
            tc.tile_pool(name="outbuf", bufs=2) as opool,
            tc.tile_pool(name="psum", bufs=8, space="PSUM") as psum1,
        ):
            xg_t, wt_t = [], []
            for s in range(ns):
                c = slot_caps[s]
                t = res.tile([P, KH, c], bf, name=f"xg{s}_t")
                nc.sync.dma_start(t[:], xgq[s].ap()[:])
                xg_t.append(t)
                w = res.tile([P, c], f32, name=f"wt{s}_t")
                nc.sync.dma_start(w[:], wtb[s].ap()[:])
                wt_t.append(w)
            xt_t = res.tile([P, KH, T], bf)
            nc.sync.dma_start(xt_t[:], xtq.ap()[:])
            sd_t = res.tile([P, KS, H], bf)
            for m in range(MH):
                nc.sync.dma_start(sd_t[:, :, m * P:(m + 1) * P], sdq.ap()[m])

            h_t = [hpool.tile([P, KI, slot_caps[s]], bf, name=f"h{s}_t", tag=f"h{s}_t")
                   for s in range(ns)]
            hs_t = hpool.tile([P, KS, T], bf)

            def silu_into(h_dst, ps_g, ps_u, n):
                sg = spool.tile([P, 512], mybir.dt.float32, tag="sg")
                nc.scalar.activation(
                    sg[:, :n], ps_g[:, :n],
                    mybir.ActivationFunctionType.Silu,
                )
                nc.vector.tensor_mul(h_dst, sg[:, :n], ps_u[:, :n])

            def g1_pr(pr):
                wt13 = wpool.tile([P, ns, 2, KH, P], bf, tag="w13", bufs=2)
                nc.sync.dma_start(wt13[:], w13p.ap()[pr])
                for s in range(ns):
                    cap = slot_caps[s]
                    psums = []
                    for pi in range(2):
                        ps = psum1.tile([P, 512], mybir.dt.float32, tag="ps",
                                        name=f"ps_g1_{pr}_{s}_{pi}")
                        for k in range(KH):
                            nc.tensor.matmul(
                                ps[:, :cap],
                                lhsT=wt13[:, s, pi, k, :],
                                rhs=xg_t[s][:, k, :cap],
                                start=(k == 0),
                                stop=(k == KH - 1),
                            )
                        psums.append(ps)
                    silu_into(h_t[s][:, pr, :cap], psums[0], psums[1], cap)

            sh_panels = {}

            def sh_g1_chunk(pair, ci):
                if pair not in sh_panels:
                    pan = wpool.tile([P, 2, KH, P], bf, tag="sgu", bufs=2,
                                     name=f"sgupan_{pair}")
                    nc.sync.dma_start(pan[:], sgup.ap()[pair])
                    sh_panels[pair] = pan
                pan = sh_panels[pair]
                o, n = ci * 512, 512
                psums = []
                for pi in range(2):
                    ps = psum1.tile([P, 512], mybir.dt.float32, tag="ps",
                                    name=f"ps_sg1_{pair}_{pi}_{ci}")
                    for k in range(KH):
                        nc.tensor.matmul(
                            ps[:, :n],
                            lhsT=pan[:, pi, k, :],
                            rhs=xt_t[:, k, o:o + n],
                            start=(k == 0),
                            stop=(k == KH - 1),
                        )
                    psums.append(ps)
                silu_into(hs_t[:, pair, o:o + n], psums[0], psums[1], n)

            def body():
                sh_panels.clear()
                sh_sched = {1: (0, 0), 3: (0, 1), 5: (1, 0), 7: (1, 1),
                            9: (2, 0), 10: (2, 1)}
                for pr in range(NPAIR):
                    g1_pr(pr)
                    if pr in sh_sched:
                        sh_g1_chunk(*sh_sched[pr])

                # phase BC: shared g2 interleaved with routed g2
                ot_sh = None
                ot_y = [None] * ns
                for m in range(MH):
                    if m % SGRP == 0:
                        ot_sh = opool.tile([P, SGRP, T], bf, tag="shout",
                                           bufs=2, name=f"ot_sh_{m}")
                    if m % YGRP == 0:
                        ot_y = [opool.tile([P, YGRP, slot_caps[s]], bf,
                                           tag=f"yout{s}", bufs=2,
                                           name=f"ot_y{s}_{m}")
                                for s in range(ns)]
                    # shared g2 panel
                    ps_sh = [psum1.tile([P, 512], mybir.dt.float32, tag="ps",
                                        name=f"ps_sh_{m}_{ci}")
                             for ci in range(2)]
                    for ci in range(2):
                        for k in range(KS):
                            nc.tensor.matmul(
                                ps_sh[ci][:],
                                lhsT=sd_t[:, k, m * P:(m + 1) * P],
                                rhs=hs_t[:, k, ci * 512:(ci + 1) * 512],
                                start=(k == 0),
                                stop=(k == KS - 1),
                            )
                    for ci in range(2):
                        nc.vector.tensor_copy(
                            ot_sh[:, m % SGRP, ci * 512:(ci + 1) * 512],
                            ps_sh[ci][:])
                    # routed g2 panels, both slots from one w2 DMA
                    w2t = wpool.tile([P, ns, KI, P], bf, tag="w2", bufs=4,
                                     name=f"w2t_{m}")
                    nc.sync.dma_start(w2t[:], w2p.ap()[m])
                    for s in range(ns):
                        cap = slot_caps[s]
                        ps = psum1.tile([P, 512], mybir.dt.float32, tag="ps",
                                        name=f"ps_g2_{m}_{s}")
                        for k in range(KI):
                            nc.tensor.matmul(
                                ps[:, :cap],
                                lhsT=w2t[:, s, k, :],
                                rhs=h_t[s][:, k, :cap],
                                start=(k == 0),
                                stop=(k == KI - 1),
                            )
                        nc.vector.tensor_mul(
                            ot_y[s][:, m % YGRP, :], ps[:, :cap], wt_t[s][:])
                    # grouped output stores on the ACT HWDGE ring
                    if m % SGRP == SGRP - 1:
                        g = m - (SGRP - 1)
                        nc.scalar.dma_start(
                            shp.ap()[:, g:g + SGRP, :], ot_sh[:])
                    if m % YGRP == YGRP - 1:
                        g = m - (YGRP - 1)
                        for s in range(ns):
                            nc.scalar.dma_start(
                                yout[s].ap()[:, g:g + YGRP, :], ot_y[s][:])

            if reps == 1:
                body()
            else:
                with tc.For_i(0, reps, 1):
                    body()

    nc.compile()
    return nc


_PROGRAM_CACHE = {}


def _get_program(CB, CS):
    key = (CB, CS)
    if key not in _PROGRAM_CACHE:
        _PROGRAM_CACHE[key] = _build_program(CB, CS)
    return _PROGRAM_CACHE[key]


def _assign_slots(counts):
    order = np.argsort(-counts, kind="stable")
    rnd8 = lambda v: max(8, int(-(-v // 8)) * 8)
    tail = [int(e) for e in order[10:] if counts[e] > 0]
    tail_tokens = int(sum(counts[e] for e in tail))
    if tail_tokens <= 512:
        CA = rnd8(-(-int(counts[order[0]]) // 2))
        instA = []
        for i in range(4):
            e = int(order[i])
            n = int(counts[e])
            n1 = (n + 1) // 2
            instA += [(e, 0, n1), (e, n1, n)]
        mid = [int(order[i]) for i in range(4, 10) if counts[order[i]] > 0]
        CB = rnd8(max(1, int(-(-sum(int(counts[e]) for e in mid) // 8))))
        while sum(-(-int(counts[e]) // CB) for e in mid) > 8:
            CB += 8
        instB = []
        for e in mid:
            n = int(counts[e])
            lo = 0
            while lo < n:
                hi = min(lo + CB, n)
                instB.append((e, lo, hi))
                lo = hi
        while len(instB) < 8:
            instB.append((int(order[0]), 0, 0))
        return CA, CB, instA, instB, tail
    big, small = order[:N_CORES], order[N_CORES:]
    CA = rnd8(int(counts[big].max()))
    CB = rnd8(int(counts[small].max())) if counts[small].max() > 0 else 0
    instA = [(int(e), 0, int(counts[e])) for e in big]
    instB = [(int(e), 0, int(counts[e])) for e in small[::-1]]
    return CA, CB, instA, instB, []


def _prepare(x, gate_weight, gate_bias, w13, w2, shared_gate_up, shared_down):
    topk_ids, topk_w = _route(x, gate_weight, gate_bias)
    flat_e = topk_ids.ravel()
    flat_w = topk_w.ravel()
    flat_t = np.repeat(np.arange(T, dtype=np.int64), TOP_K)
    idx_e = [flat_t[flat_e == e] for e in range(E)]
    w_e = [flat_w[flat_e == e] for e in range(E)]
    counts = np.array([len(i) for i in idx_e])

    CB, CS, instA, instB, tail = _assign_slots(counts)

    xt_pack = _pack_rhs(x.astype(BF16))

    packed13, packed2 = {}, {}
    in_maps, meta = [], []
    for c in range(N_CORES):
        insts = [instA[c]] + ([instB[c]] if CS else [])
        caps = [CB] + ([CS] if CS else [])
        im = {}
        cmeta = []
        p13_slots, p2_slots = [], []
        for s, ((e, lo, hi), cap) in enumerate(zip(insts, caps)):
            idx = idx_e[e][lo:hi]
            n = len(idx)
            xg = np.zeros((cap, H), dtype=BF16)
            xg[:n] = x[idx].astype(BF16)
            im[f"xgq{s}"] = _pack_rhs(xg)
            wt = np.zeros((cap,), dtype=F32)
            wt[:n] = w_e[e][lo:hi].astype(F32)
            im[f"wtb{s}"] = np.ascontiguousarray(
                np.broadcast_to(wt[None, :], (P, cap)).astype(F32))
            if e not in packed13:
                packed13[e] = _pack_lhs_panels(w13[e].astype(BF16), MW, KH)
                packed2[e] = _pack_lhs_panels(w2[e].astype(BF16), MH, KI)
            p13_slots.append(packed13[e])
            p2_slots.append(packed2[e])
            cmeta.append((s, e, idx))
        ns = len(caps)
        # [NPAIR, P, ns, 2, KH, P]: both slots' (pr, pr+NPAIR) panels per pr
        a = np.stack([np.stack([p[:NPAIR], p[NPAIR:]], axis=1)
                      for p in p13_slots], axis=1)  # [NPAIR, ns, 2, P, KH, P]
        im["w13p"] = np.ascontiguousarray(a.transpose(0, 3, 1, 2, 4, 5))
        b = np.stack(p2_slots, axis=1)               # [MH, ns, P, KI, P]
        im["w2p"] = np.ascontiguousarray(b.transpose(0, 2, 1, 3, 4))

        sh = IS // N_CORES
        lo, hi = c * sh, (c + 1) * sh
        gsl = np.zeros((SSH, H), dtype=F32)
        usl = np.zeros((SSH, H), dtype=F32)
        gsl[:hi - lo] = shared_gate_up[lo:hi]
        usl[:hi - lo] = shared_gate_up[IS + lo:IS + hi]
        sgu_pad = np.concatenate([gsl, usl], 0).astype(BF16)
        sg_panels = _pack_lhs_panels(sgu_pad, 2 * KS, KH)  # [2*KS, P, KH, P]
        sp = np.stack([sg_panels[:KS], sg_panels[KS:]], axis=1)
        im["sgup"] = np.ascontiguousarray(sp.transpose(0, 2, 1, 3, 4))
        sd_sl = np.zeros((H, SSH), dtype=F32)
        sd_sl[:, :hi - lo] = shared_down[:, lo:hi]
        im["sdq"] = _pack_lhs_panels(sd_sl.astype(BF16), MH, KS)
        im["xtq"] = xt_pack
        in_maps.append(im)
        meta.append(cmeta)

    tail_add = []
    for e in tail:
        idx = idx_e[e]
        if len(idx) == 0:
            continue
        xt32 = x[idx].astype(F32)
        gu = xt32 @ w13[e].astype(F32).T
        y = _silu_mul_np(gu) @ w2[e].astype(F32).T
        tail_add.append((idx, y * w_e[e].astype(F32)[:, None]))
    meta = {"slots": meta, "tail_add": tail_add}
    return CB, CS, in_maps, meta


def _combine(results, meta):
    # shp/y are partition-major: [P, MH, cols] -> (m*P + p, cols)
    out = np.zeros((H, T), dtype=F32)
    for c in range(N_CORES):
        s = results[c]["shp"].reshape(P, MH, T).astype(F32)
        out += s.transpose(1, 0, 2).reshape(H, T)
    out = np.ascontiguousarray(out.T)
    for c in range(N_CORES):
        r = results[c]
        for (s, e, idx) in meta["slots"][c]:
            n = len(idx)
            if n:
                y = r[f"y{s}"].astype(F32)
                y = y.reshape(P, MH, -1).transpose(1, 0, 2).reshape(H, -1)
                out[idx] += y[:, :n].T
    for idx, y in meta["tail_add"]:
        out[idx] += y
    return out


def kernel(hidden_states, gate_weight, gate_bias, w13, w2,
           shared_gate_up, shared_down):
    x = np.asarray(hidden_states, dtype=F32)
    gate_weight = np.asarray(gate_weight, dtype=F32)
    gate_bias = np.asarray(gate_bias, dtype=F32)
    w13 = np.asarray(w13, dtype=F32)
    w2 = np.asarray(w2, dtype=F32)
    shared_gate_up = np.asarray(shared_gate_up, dtype=F32)
    shared_down = np.asarray(shared_down, dtype=F32)

    CB, CS, in_maps, meta = _prepare(
        x, gate_weight, gate_bias, w13, w2, shared_gate_up, shared_down)
    nc = _get_program(CB, CS)
    res = run_bass_kernel_spmd(nc, in_maps, core_ids=list(range(N_CORES)))
    return _combine(res.results, meta)


# revision 3
# speedup vs baseline: 1.0259x; 1.0259x over previous
"""DeepseekMoE (moe_routing) Trainium2 kernel.

The v1/v2 kernels are limited by HWDGE ring occupancy (~130 DMAs/rep of
360-512KB at ~150-270 GB/s effective on a single FIFO ring). v3:
- one 2MB DMA per pr for w13 (both slots x both pair panels packed on
  the host into [NPAIR, P, ns, 2, KH, P]);
- one 720KB DMA per m for w2 (both slots packed [MH, P, ns, KI, P]);
- one 1MB DMA per shared pair ([KS, P, 2, KH, P]);
- outputs moved to the second HWDGE ring (ACT engine) and packed into
  ~0.7-1MB stores via partition-major DRAM layouts ([P, MH, T] /
  [P, MH, cap]);
- ACT Silu + single DVE mul; shp psum copies pinned to DVE so the ACT
  ring only carries output DMAs.
"""

import numpy as np
import ml_dtypes

import concourse.mybir as mybir
import concourse.tile as tile
from concourse import bacc
from concourse.bass_utils import run_bass_kernel_spmd

BF16 = ml_dtypes.bfloat16
F32 = np.float32

T, H, E, I = 1024, 2048, 16, 1408
I2 = 2 * I
IS = 2 * I
SSH = 384
TOP_K, N_GROUP, TOPK_GROUP = 4, 4, 2
ROUTED_SCALE = 2.5
N_CORES = 8
P = 128
KH = H // P
KI = I // P
MW = I2 // P
MH = H // P
NPAIR = I // P
KS = SSH // P


def _sigmoid(x):
    return 1.0 / (1.0 + np.exp(-x))


def _route(x, gate_weight, gate_bias):
    logits = x.astype(np.float64) @ gate_weight.astype(np.float64).T
    scores = _sigmoid(logits)
    choice = scores + gate_bias.astype(np.float64)[None, :]
    g = choice.reshape(T, N_GROUP, E // N_GROUP)
    top2sum = np.sort(g, axis=-1)[..., -2:].sum(-1)
    gidx = np.argsort(-top2sum, axis=-1, kind="stable")[:, :TOPK_GROUP]
    gmask = np.zeros((T, N_GROUP), bool)
    gmask[np.arange(T)[:, None], gidx] = True
    emask = np.repeat(gmask, E // N_GROUP, axis=1)
    masked = np.where(emask, choice, -np.inf)
    topk_ids = np.argsort(-masked, axis=-1, kind="stable")[:, :TOP_K]
    topk_w = np.take_along_axis(scores, topk_ids, axis=1)
    topk_w = topk_w / topk_w.sum(-1, keepdims=True) * ROUTED_SCALE
    return topk_ids.astype(np.int32), topk_w


def _silu_mul_np(gu):
    g, u = gu[:, :gu.shape[1] // 2], gu[:, gu.shape[1] // 2:]
    return (g / (1.0 + np.exp(-g))) * u


def _pack_lhs_panels(w, n_m, n_k):
    a = w.reshape(n_m, P, n_k, P)
    return np.ascontiguousarray(a.transpose(0, 3, 2, 1))


def _pack_rhs(xcols):
    a = xcols.reshape(-1, KH, P)
    return np.ascontiguousarray(a.transpose(2, 1, 0))


def _build_program(CB, CS, reps=1):
    nc = bacc.Bacc(None, target_bir_lowering=False)
    bf = mybir.dt.bfloat16
    f32 = mybir.dt.float32

    slot_caps = [c for c in (CB, CS) if c > 0]
    ns = len(slot_caps)

    w13p = nc.dram_tensor("w13p", [NPAIR, P, ns, 2, KH, P], bf,
                          kind="ExternalInput")
    w2p = nc.dram_tensor("w2p", [MH, P, ns, KI, P], bf, kind="ExternalInput")
    xgq = [nc.dram_tensor(f"xgq{s}", [P, KH, slot_caps[s]], bf, kind="ExternalInput")
           for s in range(ns)]
    wtb = [nc.dram_tensor(f"wtb{s}", [P, slot_caps[s]], f32, kind="ExternalInput")
           for s in range(ns)]
    sgup = nc.dram_tensor("sgup", [KS, P, 2, KH, P], bf, kind="ExternalInput")
    sdq = nc.dram_tensor("sdq", [MH, P, KS, P], bf, kind="ExternalInput")
    xtq = nc.dram_tensor("xtq", [P, KH, T], bf, kind="ExternalInput")

    # partition-major outputs so stores pack into multi-panel DMAs
    yout = [nc.dram_tensor(f"y{s}", [P, MH, slot_caps[s]], bf,
                           kind="ExternalOutput") for s in range(ns)]
    shp = nc.dram_tensor("shp", [P, MH, T], bf, kind="ExternalOutput")

    YGRP = 8   # y m-panels per output DMA
    SGRP = 4   # shp m-panels per output DMA

    with tile.TileContext(nc) as tc:
        with (
            tc.tile_pool(name="resident", bufs=1) as res,
            tc.tile_pool(name="wpanel", bufs=2) as wpool,
            tc.tile_pool(name="hbuf", bufs=1) as hpool,
            tc.tile_pool(name="silu", bufs=4) as spool,
            tc.tile_pool(name="outbuf", bufs=2) as opool,
            tc.tile_pool(name="psum", bufs=8, space="PSUM") as psum1,
        ):
            xg_t, wt_t = [], []
            for s in range(ns):
                c = slot_caps[s]
                t = res.tile([P, KH, c], bf, name=f"xg{s}_t")
                nc.sync.dma_start(t[:], xgq[s].ap()[:])
                xg_t.append(t)
                w = res.tile([P, c], f32, name=f"wt{s}_t")
                nc.sync.dma_start(w[:], wtb[s].ap()[:])
                wt_t.append(w)
            xt_t = res.tile([P, KH, T], bf)
            nc.sync.dma_start(xt_t[:], xtq.ap()[:])
            sd_t = res.tile([P, KS, H], bf)
            for m in range(MH):
                nc.sync.dma_start(sd_t[:, :, m * P:(m + 1) * P], sdq.ap()[m])

            h_t = [hpool.tile([P, KI, slot_caps[s]], bf, name=f"h{s}_t", tag=f"h{s}_t")
                   for s in range(ns)]
            hs_t = hpool.tile([P, KS, T], bf)

            def silu_into(h_dst, ps_g, ps_u, n):
                sg = spool.tile([P, 512], mybir.dt.float32, tag="sg")
                nc.scalar.activation(
                    sg[:, :n], ps_g[:, :n],
                    mybir.ActivationFunctionType.Silu,
                )
                nc.vector.tensor_mul(h_dst, sg[:, :n], ps_u[:, :n])

            def g1_pr(pr):
                wt13 = wpool.tile([P, ns, 2, KH, P], bf, tag="w13", bufs=2)
                nc.sync.dma_start(wt13[:], w13p.ap()[pr])
                for s in range(ns):
                    cap = slot_caps[s]
                    psums = []
                    for pi in range(2):
                        ps = psum1.tile([P, 512], mybir.dt.float32, tag="ps",
                                        name=f"ps_g1_{pr}_{s}_{pi}")
                        for k in range(KH):
                            nc.tensor.matmul(
                                ps[:, :cap],
                                lhsT=wt13[:, s, pi, k, :],
                                rhs=xg_t[s][:, k, :cap],
                                start=(k == 0),
                                stop=(k == KH - 1),
                            )
                        psums.append(ps)
                    silu_into(h_t[s][:, pr, :cap], psums[0], psums[1], cap)

            sh_panels = {}

            def sh_g1_chunk(pair, ci):
                if pair not in sh_panels:
                    pan = wpool.tile([P, 2, KH, P], bf, tag="sgu", bufs=2,
                                     name=f"sgupan_{pair}")
                    nc.sync.dma_start(pan[:], sgup.ap()[pair])
                    sh_panels[pair] = pan
                pan = sh_panels[pair]
                o, n = ci * 512, 512
                psums = []
                for pi in range(2):
                    ps = psum1.tile([P, 512], mybir.dt.float32, tag="ps",
                                    name=f"ps_sg1_{pair}_{pi}_{ci}")
                    for k in range(KH):
                        nc.tensor.matmul(
                            ps[:, :n],
                            lhsT=pan[:, pi, k, :],
                            rhs=xt_t[:, k, o:o + n],
                            start=(k == 0),
                            stop=(k == KH - 1),
                        )
                    psums.append(ps)
                silu_into(hs_t[:, pair, o:o + n], psums[0], psums[1], n)

            def body():
                sh_panels.clear()
                sh_sched = {1: (0, 0), 3: (0, 1), 5: (1, 0), 6: (1, 1),
                            8: (2, 0), 9: (2, 1)}
                for pr in range(NPAIR):
                    g1_pr(pr)
                    if pr in sh_sched:
                        sh_g1_chunk(*sh_sched[pr])

                # phase BC: shared g2 interleaved with routed g2
                ot_sh = None
                ot_y = [None] * ns
                for m in range(MH):
                    if m % SGRP == 0:
                        ot_sh = opool.tile([P, SGRP, T], bf, tag="shout",
                                           bufs=2, name=f"ot_sh_{m}")
                    if m % YGRP == 0:
                        ot_y = [opool.tile([P, YGRP, slot_caps[s]], bf,
                                           tag=f"yout{s}", bufs=2,
                                           name=f"ot_y{s}_{m}")
                                for s in range(ns)]
                    # shared g2 panel
                    ps_sh = [psum1.tile([P, 512], mybir.dt.float32, tag="ps",
                                        name=f"ps_sh_{m}_{ci}")
                             for ci in range(2)]
                    for ci in range(2):
                        for k in range(KS):
                            nc.tensor.matmul(
                                ps_sh[ci][:],
                                lhsT=sd_t[:, k, m * P:(m + 1) * P],
                                rhs=hs_t[:, k, ci * 512:(ci + 1) * 512],
                                start=(k == 0),
                                stop=(k == KS - 1),
                            )
                    for ci in range(2):
                        nc.vector.tensor_copy(
                            ot_sh[:, m % SGRP, ci * 512:(ci + 1) * 512],
                            ps_sh[ci][:])
                    # routed g2 panels, both slots from one w2 DMA
                    w2t = wpool.tile([P, ns, KI, P], bf, tag="w2", bufs=4,
                                     name=f"w2t_{m}")
                    nc.sync.dma_start(w2t[:], w2p.ap()[m])
                    for s in range(ns):
                        cap = slot_caps[s]
                        ps = psum1.tile([P, 512], mybir.dt.float32, tag="ps",
                                        name=f"ps_g2_{m}_{s}")
                        for k in range(KI):
                            nc.tensor.matmul(
                                ps[:, :cap],
                                lhsT=w2t[:, s, k, :],
                                rhs=h_t[s][:, k, :cap],
                                start=(k == 0),
                                stop=(k == KI - 1),
                            )
                        nc.vector.tensor_mul(
                            ot_y[s][:, m % YGRP, :], ps[:, :cap], wt_t[s][:])
                    # grouped output stores on the ACT HWDGE ring
                    if m % SGRP == SGRP - 1:
                        g = m - (SGRP - 1)
                        nc.scalar.dma_start(
                            shp.ap()[:, g:g + SGRP, :], ot_sh[:])
                    if m % YGRP == YGRP - 1:
                        g = m - (YGRP - 1)
                        for s in range(ns):
                            nc.scalar.dma_start(
                                yout[s].ap()[:, g:g + YGRP, :], ot_y[s][:])

            if reps == 1:
                body()
            else:
                with tc.For_i(0, reps, 1):
                    body()

    nc.compile()
    return nc


_PROGRAM_CACHE = {}


def _get_program(CB, CS):
    key = (CB, CS)
    if key not in _PROGRAM_CACHE:
        _PROGRAM_CACHE[key] = _build_program(CB, CS)
    return _PROGRAM_CACHE[key]


def _assign_slots(counts):
    order = np.argsort(-counts, kind="stable")
    rnd8 = lambda v: max(8, int(-(-v // 8)) * 8)
    tail = [int(e) for e in order[10:] if counts[e] > 0]
    tail_tokens = int(sum(counts[e] for e in tail))
    if tail_tokens <= 512:
        CA = max(8, -(-int(counts[order[0]]) // 2))
        instA = []
        for i in range(4):
            e = int(order[i])
            n = int(counts[e])
            n1 = (n + 1) // 2
            instA += [(e, 0, n1), (e, n1, n)]
        mid = [int(order[i]) for i in range(4, 10) if counts[order[i]] > 0]
        # split the largest mid chunks in half until there are 8 chunks;
        # CB is then the max chunk (no rounding - exact capacity)
        chunks = [(int(counts[e]), e, 0, int(counts[e])) for e in mid]
        while len(chunks) < 8:
            chunks.sort(reverse=True)
            sz, e, lo, hi = chunks[0]
            mid_pt = lo + (sz + 1) // 2
            chunks[0] = (mid_pt - lo, e, lo, mid_pt)
            chunks.append((hi - mid_pt, e, mid_pt, hi))
        CB = max(8, max(c[0] for c in chunks))
        instB = [(e, lo, hi) for (sz, e, lo, hi) in chunks]
        return CA, CB, instA, instB, tail
    big, small = order[:N_CORES], order[N_CORES:]
    CA = rnd8(int(counts[big].max()))
    CB = rnd8(int(counts[small].max())) if counts[small].max() > 0 else 0
    instA = [(int(e), 0, int(counts[e])) for e in big]
    instB = [(int(e), 0, int(counts[e])) for e in small[::-1]]
    return CA, CB, instA, instB, []


def _prepare(x, gate_weight, gate_bias, w13, w2, shared_gate_up, shared_down):
    topk_ids, topk_w = _route(x, gate_weight, gate_bias)
    flat_e = topk_ids.ravel()
    flat_w = topk_w.ravel()
    flat_t = np.repeat(np.arange(T, dtype=np.int64), TOP_K)
    idx_e = [flat_t[flat_e == e] for e in range(E)]
    w_e = [flat_w[flat_e == e] for e in range(E)]
    counts = np.array([len(i) for i in idx_e])

    CB, CS, instA, instB, tail = _assign_slots(counts)

    xt_pack = _pack_rhs(x.astype(BF16))

    packed13, packed2 = {}, {}
    in_maps, meta = [], []
    for c in range(N_CORES):
        insts = [instA[c]] + ([instB[c]] if CS else [])
        caps = [CB] + ([CS] if CS else [])
        im = {}
        cmeta = []
        p13_slots, p2_slots = [], []
        for s, ((e, lo, hi), cap) in enumerate(zip(insts, caps)):
            idx = idx_e[e][lo:hi]
            n = len(idx)
            xg = np.zeros((cap, H), dtype=BF16)
            xg[:n] = x[idx].astype(BF16)
            im[f"xgq{s}"] = _pack_rhs(xg)
            wt = np.zeros((cap,), dtype=F32)
            wt[:n] = w_e[e][lo:hi].astype(F32)
            im[f"wtb{s}"] = np.ascontiguousarray(
                np.broadcast_to(wt[None, :], (P, cap)).astype(F32))
            if e not in packed13:
                packed13[e] = _pack_lhs_panels(w13[e].astype(BF16), MW, KH)
                packed2[e] = _pack_lhs_panels(w2[e].astype(BF16), MH, KI)
            p13_slots.append(packed13[e])
            p2_slots.append(packed2[e])
            cmeta.append((s, e, idx))
        ns = len(caps)
        # [NPAIR, P, ns, 2, KH, P]: both slots' (pr, pr+NPAIR) panels per pr
        a = np.stack([np.stack([p[:NPAIR], p[NPAIR:]], axis=1)
                      for p in p13_slots], axis=1)  # [NPAIR, ns, 2, P, KH, P]
        im["w13p"] = np.ascontiguousarray(a.transpose(0, 3, 1, 2, 4, 5))
        b = np.stack(p2_slots, axis=1)               # [MH, ns, P, KI, P]
        im["w2p"] = np.ascontiguousarray(b.transpose(0, 2, 1, 3, 4))

        sh = IS // N_CORES
        lo, hi = c * sh, (c + 1) * sh
        gsl = np.zeros((SSH, H), dtype=F32)
        usl = np.zeros((SSH, H), dtype=F32)
        gsl[:hi - lo] = shared_gate_up[lo:hi]
        usl[:hi - lo] = shared_gate_up[IS + lo:IS + hi]
        sgu_pad = np.concatenate([gsl, usl], 0).astype(BF16)
        sg_panels = _pack_lhs_panels(sgu_pad, 2 * KS, KH)  # [2*KS, P, KH, P]
        sp = np.stack([sg_panels[:KS], sg_panels[KS:]], axis=1)
        im["sgup"] = np.ascontiguousarray(sp.transpose(0, 2, 1, 3, 4))
        sd_sl = np.zeros((H, SSH), dtype=F32)
        sd_sl[:, :hi - lo] = shared_down[:, lo:hi]
        im["sdq"] = _pack_lhs_panels(sd_sl.astype(BF16), MH, KS)
        im["xtq"] = xt_pack
        in_maps.append(im)
        meta.append(cmeta)

    tail_add = []
    for e in tail:
        idx = idx_e[e]
        if len(idx) == 0:
            continue
        xt32 = x[idx].astype(F32)
        gu = xt32 @ w13[e].astype(F32).T
        y = _silu_mul_np(gu) @ w2[e].astype(F32).T
        tail_add.append((idx, y * w_e[e].astype(F32)[:, None]))
    meta = {"slots": meta, "tail_add": tail_add}
    return CB, CS, in_maps, meta


def _combine(results, meta):
    # shp/y are partition-major: [P, MH, cols] -> (m*P + p, cols)
    out = np.zeros((H, T), dtype=F32)
    for c in range(N_CORES):
        s = results[c]["shp"].reshape(P, MH, T).astype(F32)
        out += s.transpose(1, 0, 2).reshape(H, T)
    out = np.ascontiguousarray(out.T)
    for c in range(N_CORES):
        r = results[c]
        for (s, e, idx) in meta["slots"][c]:
            n = len(idx)
            if n:
                y = r[f"y{s}"].astype(F32)
                y = y.reshape(P, MH, -1).transpose(1, 0, 2).reshape(H, -1)
                out[idx] += y[:, :n].T
    for idx, y in meta["tail_add"]:
        out[idx] += y
    return out


def kernel(hidden_states, gate_weight, gate_bias, w13, w2,
           shared_gate_up, shared_down):
    x = np.asarray(hidden_states, dtype=F32)
    gate_weight = np.asarray(gate_weight, dtype=F32)
    gate_bias = np.asarray(gate_bias, dtype=F32)
    w13 = np.asarray(w13, dtype=F32)
    w2 = np.asarray(w2, dtype=F32)
    shared_gate_up = np.asarray(shared_gate_up, dtype=F32)
    shared_down = np.asarray(shared_down, dtype=F32)

    CB, CS, in_maps, meta = _prepare(
        x, gate_weight, gate_bias, w13, w2, shared_gate_up, shared_down)
    nc = _get_program(CB, CS)
    res = run_bass_kernel_spmd(nc, in_maps, core_ids=list(range(N_CORES)))
    return _combine(res.results, meta)
